# revision 19
# baseline (speedup 1.0000x reference)
"""Trainium2 Bass kernel for nn_DisLoss: loss = sum(x * dist_to_argmax(x)) / b.

x: (128, 512, 512) f32. Data-parallel over 8 NeuronCores: 16 images per core.
Per image on-device:
  1. DVE 3D reduce-max -> per-row maxes (partition p holds rows 4p..4p+3)
  2. PE transpose + tiny DVE reduce -> global max M; PE ones-matmuls do all
     cross-partition sums/broadcasts (GPSIMD stays on its default ucode lib)
  3. indirect-DMA gather of the winning row spread over 16 partitions x 64
     (row appears twice); masked iota-dot -> column -cx
  4. ACT: colsq=(j-cx)^2, rowsq=(r-cy)^2, dist_t=sqrt(colsq+rowsq_t)
  5. z = x*dist: chunk t=0 fused mul+accum on DVE -> partials[:, b];
     chunks t=1..3 multiplied on GPSIMD, column-summed into one PSUM row by
     accumulating PE matmuls (lhsT = ones column)
Host sums partials + colsums in float64 and divides by b.
"""

import numpy as np

B_FULL = 128
H = 512
W = 512
N_CORES = 8
B_CORE = B_FULL // N_CORES  # 16 images per core
T = 4                       # rows per partition
ROWP = 8                    # partitions for the gathered-row scan
ROWF = 64                   # free elems per partition in the row scan

_CACHE = {}


def _consts():
    cols = np.broadcast_to(np.arange(W, dtype=np.float32), (128, W)).copy()
    r4c = (4 * np.arange(128, dtype=np.float32)[:, None]
           + np.arange(T, dtype=np.float32)[None, :]).copy()
    r4enc8 = (8.0 * (4096.0 - r4c)).copy()
    lin = (64 * np.arange(ROWP, dtype=np.float32)[:, None]
           + np.arange(ROWF, dtype=np.float32)[None, :])
    colenc = (512.0 - lin).astype(np.float32)
    pidx8 = np.arange(ROWP, dtype=np.float32)[:, None].copy()
    ones2 = np.ones((1, 2), dtype=np.float32)
    pec = np.concatenate([np.eye(128, dtype=np.float32),
                          np.ones((128, 128), dtype=np.float32)], axis=1)
    return {"COLS": cols, "R4C": r4c, "R4ENC8": r4enc8, "COLENC": colenc,
            "PIDX8": pidx8, "ONES2": ones2, "PEC": pec}


def build_program(debug=False, b_core=None, no_indirect=False,
                  no_psum_acc=False, repeat=1):
    import concourse.bass as bass
    import concourse.bacc as bacc
    import concourse.mybir as mybir
    from concourse.tile import TileContext

    nb = b_core or B_CORE
    f32 = mybir.dt.float32
    u32 = mybir.dt.uint32
    Alu = mybir.AluOpType
    Act = mybir.ActivationFunctionType

    nc = bacc.Bacc("TRN2", target_bir_lowering=False, debug=False)

    x_d = nc.dram_tensor("x", [nb, H, W], f32, kind="ExternalInput")
    cols_d = nc.dram_tensor("COLS", [128, W], f32, kind="ExternalInput")
    r4c_d = nc.dram_tensor("R4C", [128, T], f32, kind="ExternalInput")
    r4enc8_d = nc.dram_tensor("R4ENC8", [128, T], f32, kind="ExternalInput")
    pidx8_d = nc.dram_tensor("PIDX8", [ROWP, 1], f32, kind="ExternalInput")
    colenc_d = nc.dram_tensor("COLENC", [ROWP, ROWF], f32, kind="ExternalInput")
    ones2_d = nc.dram_tensor("ONES2", [1, 2], f32, kind="ExternalInput")
    pec_d = nc.dram_tensor("PEC", [128, 256], f32, kind="ExternalInput")
    out_d = nc.dram_tensor("partials", [128, nb], f32, kind="ExternalOutput")
    csum_d = nc.dram_tensor("colsums", [1, W], f32, kind="ExternalOutput")
    if debug:
        dbg_d = nc.dram_tensor("dbg", [1, 4 * nb], f32, kind="ExternalOutput")

    x_ap = x_d.ap()
    x_rows = x_ap.rearrange("b h (s i) -> (b h s) i", i=ROWF)  # 64-elem sub-rows

    with TileContext(nc) as tc:
        with (
            tc.tile_pool(name="consts", bufs=1) as consts,
            tc.tile_pool(name="xs", bufs=6) as xs,
            tc.tile_pool(name="work", bufs=6) as work,
            tc.tile_pool(name="small", bufs=6) as small,
            tc.tile_pool(name="rows", bufs=4) as rows,
            tc.tile_pool(name="ps", bufs=3, space="PSUM") as ps,
            tc.tile_pool(name="pacc", bufs=1, space="PSUM") as pacc,
        ):
            cols_t = consts.tile([128, W], f32)
            nc.sync.dma_start(out=cols_t, in_=cols_d.ap())
            r4c_t = consts.tile([128, T], f32)
            nc.sync.dma_start(out=r4c_t, in_=r4c_d.ap())
            r4enc8_t = consts.tile([128, T], f32)
            nc.sync.dma_start(out=r4enc8_t, in_=r4enc8_d.ap())
            pidx8_t = consts.tile([ROWP, 1], f32)
            nc.sync.dma_start(out=pidx8_t, in_=pidx8_d.ap())
            colenc_t = consts.tile([ROWP, ROWF], f32)
            nc.sync.dma_start(out=colenc_t, in_=colenc_d.ap())
            ones2_t = consts.tile([1, 2], f32)
            nc.sync.dma_start(out=ones2_t, in_=ones2_d.ap())
            pec_t = consts.tile([128, 256], f32)
            nc.sync.dma_start(out=pec_t, in_=pec_d.ap())
            ident_t = pec_t[:, 0:128]
            ones_t = pec_t[:, 128:256]

            # prime PE on the const tile so later PE ops carry only one wait
            prime_ps = ps.tile([1, 128], f32, tag="tpose")
            nc.tensor.transpose(prime_ps, pec_t[:, 128:129], ident_t)

            partials_t = consts.tile([128, nb], f32)
            colsums_ps = pacc.tile([1, W], f32)
            if debug:
                dbg_t = consts.tile([1, 4 * nb], f32)
                nc.vector.memset(dbg_t, 0.0)

            import itertools
            for rep, b in itertools.product(range(repeat), range(nb)):
                # --- load image b: partition p holds rows 4p..4p+3 ---
                x_t = xs.tile([128, T, W], f32)
                nc.sync.dma_start(
                    out=x_t,
                    in_=x_ap[b].rearrange("(p t) w -> p t w", p=128),
                )

                # --- per-row maxes -> per-partition max -> global max M ---
                rowmax = small.tile([128, T], f32)
                nc.vector.reduce_max(rowmax, x_t, axis=mybir.AxisListType.X)
                pmax = small.tile([128, 1], f32)
                nc.vector.reduce_max(pmax, rowmax, axis=mybir.AxisListType.X)
                pmax_ps = ps.tile([1, 128], f32, tag="tpose")
                nc.tensor.transpose(pmax_ps, pmax, ident_t)
                m_sb = small.tile([1, 1], f32)
                nc.vector.reduce_max(m_sb, pmax_ps, axis=mybir.AxisListType.X)
                sc_ps = ps.tile([128, 3], f32, tag="sc")
                m_ps = sc_ps[:, 0:1]
                nc.tensor.matmul(m_ps, ones_t[0:1, :], m_sb)

                # --- locate row: max of (rowmax==M)*(8*(4096-r)) -> first row ---
                e4 = small.tile([128, T], f32)
                junk1 = small.tile([128, 1], f32)
                nc.vector.scalar_tensor_tensor(
                    e4, rowmax, m_ps, r4enc8_t,
                    op0=Alu.is_equal, op1=Alu.mult, accum_out=junk1)
                epmax = small.tile([128, 1], f32)
                nc.vector.reduce_max(epmax, e4, axis=mybir.AxisListType.X)
                ep_ps = ps.tile([1, 128], f32, tag="tpose")
                nc.tensor.transpose(ep_ps, epmax, ident_t)
                e_sb = small.tile([1, 1], f32)
                nc.vector.reduce_max(e_sb, ep_ps, axis=mybir.AxisListType.X)
                nc.tensor.matmul(sc_ps[:, 1:2], ones_t[0:1, :], e_sb)

                # --- gather the winning row, 64 elems per partition ---
                cyu = small.tile([ROWP, 1], u32)
                nc.vector.tensor_scalar(
                    cyu, pidx8_t, sc_ps[0:ROWP, 1:2], float(8 * 4096 + 8 * H * b),
                    op0=Alu.subtract, op1=Alu.add)
                rowbuf = rows.tile([ROWP, ROWF], f32)
                if no_indirect:
                    nc.sync.dma_start(out=rowbuf, in_=x_ap[b, 0:ROWP, 0:ROWF])
                else:
                    nc.gpsimd.indirect_dma_start(
                        out=rowbuf,
                        out_offset=None,
                        in_=x_rows,
                        in_offset=bass.IndirectOffsetOnAxis(ap=cyu[:], axis=0),
                    )

                # --- locate column: max of (row==M)*(512-j) -> first col ---
                ohr = rows.tile([ROWP, ROWF], f32)
                junkr = small.tile([ROWP, 1], f32)
                nc.vector.scalar_tensor_tensor(
                    ohr, rowbuf, m_ps[0:ROWP, 0:1], colenc_t,
                    op0=Alu.is_equal, op1=Alu.mult, accum_out=junkr)
                ecp = small.tile([ROWP, 1], f32)
                nc.vector.reduce_max(ecp, ohr, axis=mybir.AxisListType.X)
                ec_ps = ps.tile([1, ROWP], f32, tag="tpose")
                nc.tensor.transpose(ec_ps, ecp, ident_t[0:ROWP, 0:ROWP])
                ec_sb = small.tile([1, 1], f32)
                nc.vector.reduce_max(ec_sb, ec_ps, axis=mybir.AxisListType.X)
                nc.tensor.matmul(sc_ps[:, 2:3], ones_t[0:1, :], ec_sb)
                nsb = small.tile([128, 2], f32)
                nc.vector.tensor_copy(nsb, sc_ps[:, 1:3])
                negcy_sb = small.tile([128, 1], f32)
                nc.vector.tensor_scalar(
                    negcy_sb, nsb[:, 0:1], 0.125, -4096.0,
                    op0=Alu.mult, op1=Alu.add)
                negcx_sb = small.tile([128, 1], f32)
                nc.vector.tensor_scalar(
                    negcx_sb, nsb[:, 1:2], -512.0, None, op0=Alu.add)

                # --- distances ---
                colsq = work.tile([128, W], f32)
                nc.scalar.activation(colsq, cols_t, Act.Square,
                                     bias=negcx_sb)
                rowsq = small.tile([128, T], f32)
                nc.scalar.activation(rowsq, r4c_t, Act.Square,
                                     bias=negcy_sb)

                for t in range(T):
                    dist_t = work.tile([128, W], f32, tag="dist")
                    nc.scalar.activation(
                        dist_t, colsq, Act.Sqrt, bias=rowsq[:, t:t + 1])
                    if t == 0:
                        z_t = work.tile([128, W], f32, tag="zd")
                        nc.vector.scalar_tensor_tensor(
                            z_t, dist_t, 1.0, x_t[:, t, :],
                            op0=Alu.mult, op1=Alu.mult,
                            accum_out=partials_t[:, b:b + 1])
                    else:
                        z_t = work.tile([128, W], f32, tag="zg")
                        nc.gpsimd.tensor_tensor(
                            z_t, dist_t, x_t[:, t, :], op=Alu.mult)
                        if no_psum_acc:
                            nc.tensor.matmul(
                                colsums_ps, ones_t[:, 0:1], z_t,
                                start=True, stop=True,
                                skip_group_check=True)
                        else:
                            nc.tensor.matmul(
                                colsums_ps, ones_t[:, 0:1], z_t,
                                start=(b == 0 and t == 1),
                                stop=(b == nb - 1 and t == T - 1),
                                skip_group_check=True)

                if debug:
                    nc.vector.tensor_scalar(
                        dbg_t[0:1, 4 * b:4 * b + 1], ones2_t[0:1, 0:1],
                        m_ps[0:1, 0:1], None, op0=Alu.mult)
                    nc.vector.tensor_scalar(
                        dbg_t[0:1, 4 * b + 1:4 * b + 2], ones2_t[0:1, 0:1],
                        nsb[0:1, 0:1], None, op0=Alu.mult)
                    nc.vector.tensor_scalar(
                        dbg_t[0:1, 4 * b + 2:4 * b + 3], ones2_t[0:1, 0:1],
                        nsb[0:1, 1:2], None, op0=Alu.mult)

            colsums_sb = consts.tile([1, W], f32)
            nc.vector.tensor_copy(colsums_sb, colsums_ps)
            nc.sync.dma_start(out=out_d.ap(), in_=partials_t)
            nc.sync.dma_start(out=csum_d.ap(), in_=colsums_sb)
            if debug:
                nc.sync.dma_start(out=dbg_d.ap(), in_=dbg_t)

    nc.compile()
    return nc


def kernel(x: np.ndarray) -> np.ndarray:
    from concourse import bass_utils

    key = "nc"
    if key not in _CACHE:
        _CACHE[key] = build_program(debug=False)
    nc = _CACHE[key]

    x = np.ascontiguousarray(x, dtype=np.float32)
    shards = x.reshape(N_CORES, B_CORE, H, W)
    consts = _consts()
    in_maps = [dict(consts, x=shards[i]) for i in range(N_CORES)]
    res = bass_utils.run_bass_kernel_spmd(
        nc, in_maps, core_ids=list(range(N_CORES)))
    total = 0.0
    for r in res.results:
        total += r["partials"].astype(np.float64).sum()
        total += r["colsums"].astype(np.float64).sum()
    return np.float32(total / B_FULL)


# revision 25
# speedup vs baseline: 1.1302x; 1.1302x over previous
"""Trainium2 Bass kernel for nn_DisLoss: loss = sum(x * dist_to_argmax(x)) / b.

x: (128, 512, 512) f32. Data-parallel over 8 NeuronCores: 16 images per core.
Per image on-device:
  1. DVE 3D reduce-max -> per-row maxes (partition p holds rows 4p..4p+3)
  2. PE transpose + tiny DVE reduce -> global max M; PE ones-matmuls do all
     cross-partition sums/broadcasts (GPSIMD stays on its default ucode lib)
  3. indirect-DMA gather of the winning row spread over 16 partitions x 64
     (row appears twice); masked iota-dot -> column -cx
  4. ACT: colsq=(j-cx)^2, rowsq=(r-cy)^2, dist_t=sqrt(colsq+rowsq_t)
  5. z = x*dist: chunk t=0 fused mul+accum on DVE -> partials[:, b];
     chunks t=1..3 multiplied on GPSIMD, column-summed into one PSUM row by
     accumulating PE matmuls (lhsT = ones column)
Host sums partials + colsums in float64 and divides by b.
"""

import numpy as np

B_FULL = 128
H = 512
W = 512
N_CORES = 8
B_CORE = B_FULL // N_CORES  # 16 images per core
T = 4                       # rows per partition
ROWP = 32                   # partitions for the gathered-row scan
ROWF = 16                   # free elems per partition in the row scan

_CACHE = {}


def _consts():
    cols = np.broadcast_to(np.arange(W, dtype=np.float32), (128, W)).copy()
    r4c = (4 * np.arange(128, dtype=np.float32)[:, None]
           + np.arange(T, dtype=np.float32)[None, :]).copy()
    r4enc8 = (32.0 * (4096.0 - r4c)).copy()
    lin = (ROWF * np.arange(ROWP, dtype=np.float32)[:, None]
           + np.arange(ROWF, dtype=np.float32)[None, :])
    colenc = (512.0 - lin).astype(np.float32)
    pidx8 = np.arange(ROWP, dtype=np.float32)[:, None].copy()
    ones2 = np.ones((1, 2), dtype=np.float32)
    pec = np.concatenate([np.eye(128, dtype=np.float32),
                          np.ones((128, 128), dtype=np.float32)], axis=1)
    return {"COLS": cols, "R4C": r4c, "R4ENC8": r4enc8, "COLENC": colenc,
            "PIDX8": pidx8, "ONES2": ones2, "PEC": pec}


def build_program(debug=False, b_core=None, no_indirect=False,
                  no_psum_acc=False, repeat=1):
    import concourse.bass as bass
    import concourse.bacc as bacc
    import concourse.mybir as mybir
    from concourse.tile import TileContext

    nb = b_core or B_CORE
    f32 = mybir.dt.float32
    u32 = mybir.dt.uint32
    Alu = mybir.AluOpType
    Act = mybir.ActivationFunctionType

    nc = bacc.Bacc("TRN2", target_bir_lowering=False, debug=False)

    x_d = nc.dram_tensor("x", [nb, H, W], f32, kind="ExternalInput")
    cols_d = nc.dram_tensor("COLS", [128, W], f32, kind="ExternalInput")
    r4c_d = nc.dram_tensor("R4C", [128, T], f32, kind="ExternalInput")
    r4enc8_d = nc.dram_tensor("R4ENC8", [128, T], f32, kind="ExternalInput")
    pidx8_d = nc.dram_tensor("PIDX8", [ROWP, 1], f32, kind="ExternalInput")
    colenc_d = nc.dram_tensor("COLENC", [ROWP, ROWF], f32, kind="ExternalInput")
    ones2_d = nc.dram_tensor("ONES2", [1, 2], f32, kind="ExternalInput")
    pec_d = nc.dram_tensor("PEC", [128, 256], f32, kind="ExternalInput")
    out_d = nc.dram_tensor("partials", [128, nb], f32, kind="ExternalOutput")
    csum_d = nc.dram_tensor("colsums", [1, W], f32, kind="ExternalOutput")
    if debug:
        dbg_d = nc.dram_tensor("dbg", [1, 4 * nb], f32, kind="ExternalOutput")

    x_ap = x_d.ap()
    x_rows = x_ap.rearrange("b h (s i) -> (b h s) i", i=ROWF)  # 64-elem sub-rows

    with TileContext(nc) as tc:
        with (
            tc.tile_pool(name="consts", bufs=1) as consts,
            tc.tile_pool(name="xs", bufs=8) as xs,
            tc.tile_pool(name="work", bufs=8) as work,
            tc.tile_pool(name="small", bufs=8) as small,
            tc.tile_pool(name="rows", bufs=8) as rows,
            tc.tile_pool(name="ps", bufs=3, space="PSUM") as ps,
            tc.tile_pool(name="pacc", bufs=1, space="PSUM") as pacc,
        ):
            cols_t = consts.tile([128, W], f32)
            nc.sync.dma_start(out=cols_t, in_=cols_d.ap())
            r4c_t = consts.tile([128, T], f32)
            nc.sync.dma_start(out=r4c_t, in_=r4c_d.ap())
            r4enc8_t = consts.tile([128, T], f32)
            nc.sync.dma_start(out=r4enc8_t, in_=r4enc8_d.ap())
            pidx8_t = consts.tile([ROWP, 1], f32)
            nc.sync.dma_start(out=pidx8_t, in_=pidx8_d.ap())
            colenc_t = consts.tile([ROWP, ROWF], f32)
            nc.sync.dma_start(out=colenc_t, in_=colenc_d.ap())
            ones2_t = consts.tile([1, 2], f32)
            nc.sync.dma_start(out=ones2_t, in_=ones2_d.ap())
            pec_t = consts.tile([128, 256], f32)
            nc.sync.dma_start(out=pec_t, in_=pec_d.ap())
            ident_t = pec_t[:, 0:128]
            ones_t = pec_t[:, 128:256]

            # prime PE on the const tile so later PE ops carry only one wait
            prime_ps = ps.tile([1, 128], f32, tag="tpose")
            nc.tensor.transpose(prime_ps, pec_t[:, 128:129], ident_t)

            partials_t = consts.tile([128, nb], f32)
            colsums_ps = pacc.tile([1, W], f32)
            if debug:
                dbg_t = consts.tile([1, 4 * nb], f32)
                nc.vector.memset(dbg_t, 0.0)

            def s0(st):
                b = st["b"]
                x_t = xs.tile([128, T, W], f32)
                nc.sync.dma_start(
                    out=x_t,
                    in_=x_ap[b].rearrange("(p t) w -> p t w", p=128),
                )
                rowmax = small.tile([128, T], f32)
                nc.vector.reduce_max(rowmax, x_t, axis=mybir.AxisListType.X)
                pmax = small.tile([128, 1], f32)
                nc.vector.reduce_max(pmax, rowmax, axis=mybir.AxisListType.X)
                pmax_ps = ps.tile([1, 128], f32, tag="tpose")
                nc.tensor.transpose(pmax_ps, pmax, ident_t)
                st.update(x_t=x_t, rowmax=rowmax, pmax_ps=pmax_ps)

            def s1(st):
                m_sb = small.tile([1, 1], f32)
                nc.vector.reduce_max(m_sb, st["pmax_ps"],
                                     axis=mybir.AxisListType.X)
                sc2 = ps.tile([128, 2], f32, tag="sc")
                nc.tensor.matmul(sc2[:, 0:1], ones_t[0:1, :], m_sb)
                msb128 = small.tile([128, 1], f32)
                nc.vector.tensor_copy(msb128, sc2[:, 0:1])
                # locate row: max of (rowmax==M)*(32*(4096-r))
                e4 = small.tile([128, T], f32)
                junk1 = small.tile([128, 1], f32)
                nc.vector.scalar_tensor_tensor(
                    e4, st["rowmax"], sc2[:, 0:1], r4enc8_t,
                    op0=Alu.is_equal, op1=Alu.mult, accum_out=junk1)
                epmax = small.tile([128, 1], f32)
                nc.vector.reduce_max(epmax, e4, axis=mybir.AxisListType.X)
                ep_ps = ps.tile([1, 128], f32, tag="tpose")
                nc.tensor.transpose(ep_ps, epmax, ident_t)
                st.update(sc2=sc2, ep_ps=ep_ps, msb128=msb128)

            def s2(st):
                b = st["b"]
                e_sb = small.tile([1, 1], f32)
                nc.vector.reduce_max(e_sb, st["ep_ps"],
                                     axis=mybir.AxisListType.X)
                sc2 = st["sc2"]
                nc.tensor.matmul(sc2[:, 1:2], ones_t[0:1, :], e_sb)
                # gather offsets: p + 32*cy + 32*512*b
                cyu = small.tile([ROWP, 1], u32)
                nc.vector.tensor_scalar(
                    cyu, pidx8_t, sc2[0:ROWP, 1:2],
                    float(32 * 4096 + ROWP * H * b),
                    op0=Alu.subtract, op1=Alu.add)
                negcy_sb = small.tile([128, 1], f32)
                nc.vector.tensor_scalar(
                    negcy_sb, sc2[:, 1:2], 1.0 / 32.0, -4096.0,
                    op0=Alu.mult, op1=Alu.add)
                st.update(cyu=cyu, negcy_sb=negcy_sb)

            def s3(st):
                b = st["b"]
                rowbuf = rows.tile([ROWP, ROWF], f32)
                if no_indirect:
                    nc.sync.dma_start(out=rowbuf, in_=x_ap[b, 0:ROWP, 0:ROWF])
                else:
                    nc.gpsimd.indirect_dma_start(
                        out=rowbuf,
                        out_offset=None,
                        in_=x_rows,
                        in_offset=bass.IndirectOffsetOnAxis(
                            ap=st["cyu"][:], axis=0),
                    )
                st.update(rowbuf=rowbuf)

            def s4(st):
                # locate column: max of (row==M)*(512-j), DVE-only 32-max
                ohr = rows.tile([ROWP, ROWF], f32)
                junkr = small.tile([ROWP, 1], f32)
                nc.vector.scalar_tensor_tensor(
                    ohr, st["rowbuf"], st["msb128"][0:ROWP, 0:1], colenc_t,
                    op0=Alu.is_equal, op1=Alu.mult, accum_out=junkr)
                ecp = small.tile([ROWP, 1], f32)
                nc.vector.reduce_max(ecp, ohr, axis=mybir.AxisListType.X)
                ecb = small.tile([ROWP, ROWP], f32)
                nc.vector.tensor_scalar(
                    ecb, ident_t[0:ROWP, 0:ROWP], 0.0, ecp,
                    op0=Alu.mult, op1=Alu.add)
                ec32 = small.tile([ROWP, 1], f32)
                nc.vector.tensor_reduce(
                    ec32, ecb, axis=mybir.AxisListType.X,
                    op=Alu.max, apply_transpose=True)
                scx = ps.tile([128, 2], f32, tag="sc")
                nc.tensor.matmul(scx[:, 0:1], ones_t[0:1, :], ec32[0:1, 0:1])
                negcx_sb = small.tile([128, 1], f32)
                nc.vector.tensor_scalar(
                    negcx_sb, scx[:, 0:1], -512.0, None, op0=Alu.add)
                st.update(negcx_sb=negcx_sb)
                if debug:
                    b = st["b"]
                    nc.vector.tensor_scalar(
                        dbg_t[0:1, 4 * b:4 * b + 1], ones2_t[0:1, 0:1],
                        st["msb128"][0:1, 0:1], None, op0=Alu.mult)
                    nc.vector.tensor_scalar(
                        dbg_t[0:1, 4 * b + 1:4 * b + 2], ones2_t[0:1, 0:1],
                        st["negcy_sb"][0:1, 0:1], None, op0=Alu.mult)
                    nc.vector.tensor_scalar(
                        dbg_t[0:1, 4 * b + 2:4 * b + 3], ones2_t[0:1, 0:1],
                        negcx_sb[0:1, 0:1], None, op0=Alu.mult)

            def s5(st):
                x_t = st["x_t"]
                colsq = work.tile([128, W], f32)
                nc.scalar.activation(colsq, cols_t, Act.Square,
                                     bias=st["negcx_sb"])
                rowsq = small.tile([128, T], f32)
                nc.scalar.activation(rowsq, r4c_t, Act.Square,
                                     bias=st["negcy_sb"])
                dists = []
                zs = []
                for t in range(T):
                    dist_t = work.tile([128, W], f32, tag="dist")
                    nc.scalar.activation(
                        dist_t, colsq, Act.Sqrt, bias=rowsq[:, t:t + 1])
                    dists.append(dist_t)
                    if t > 0:
                        z_t = work.tile([128, W], f32, tag="zg")
                        nc.gpsimd.tensor_tensor(
                            z_t, dist_t, x_t[:, t, :], op=Alu.mult)
                        zs.append(z_t)
                st.update(dist0=dists[0], zs=zs)

            def s6(st, first, last):
                x_t, b = st["x_t"], st["b"]
                z_t = work.tile([128, W], f32, tag="zd")
                nc.vector.scalar_tensor_tensor(
                    z_t, st["dist0"], 1.0, x_t[:, 0, :],
                    op0=Alu.mult, op1=Alu.mult,
                    accum_out=partials_t[:, b:b + 1])
                for t in range(1, T):
                    nc.tensor.matmul(
                        colsums_ps, ones_t[:, 0:1], st["zs"][t - 1],
                        start=(no_psum_acc or (first and t == 1)),
                        stop=(no_psum_acc or (last and t == T - 1)),
                        skip_group_check=True)

            stages = [s0, s1, s2, s3, s4, s5]
            todo = [(rep, b) for rep in range(repeat) for b in range(nb)]
            n = len(todo)
            states = {}
            NS = 7
            for i in range(n + NS - 1):
                # oldest stage first so each sequencer sees only ready work
                j = i - (NS - 1)
                if 0 <= j < n:
                    s6(states.pop(j), first=(j == 0), last=(j == n - 1))
                for k in range(NS - 2, -1, -1):
                    j = i - k
                    if 0 <= j < n:
                        if k == 0:
                            states[j] = {"b": todo[j][1], "rep": todo[j][0]}
                        stages[k](states[j])

            colsums_sb = consts.tile([1, W], f32)
            nc.vector.tensor_copy(colsums_sb, colsums_ps)
            nc.sync.dma_start(out=out_d.ap(), in_=partials_t)
            nc.sync.dma_start(out=csum_d.ap(), in_=colsums_sb)
            if debug:
                nc.sync.dma_start(out=dbg_d.ap(), in_=dbg_t)

    nc.compile()
    return nc


def kernel(x: np.ndarray) -> np.ndarray:
    from concourse import bass_utils

    key = "nc"
    if key not in _CACHE:
        _CACHE[key] = build_program(debug=False)
    nc = _CACHE[key]

    x = np.ascontiguousarray(x, dtype=np.float32)
    shards = x.reshape(N_CORES, B_CORE, H, W)
    consts = _consts()
    in_maps = [dict(consts, x=shards[i]) for i in range(N_CORES)]
    res = bass_utils.run_bass_kernel_spmd(
        nc, in_maps, core_ids=list(range(N_CORES)))
    total = 0.0
    for r in res.results:
        total += r["partials"].astype(np.float64).sum()
        total += r["colsums"].astype(np.float64).sum()
    return np.float32(total / B_FULL)


# revision 34
# speedup vs baseline: 276.9357x; 245.0388x over previous
"""Trainium2 Bass kernel for nn_DisLoss: loss = sum(x * dist_to_argmax(x)) / b.

x: (128, 512, 512) f32. Data-parallel over 8 NeuronCores: 16 images per core.
Per image on-device:
  1. DVE 3D reduce-max -> per-row maxes (partition p holds rows 4p..4p+3)
  2. PE transpose + tiny DVE reduce -> global max M; PE ones-matmuls do all
     cross-partition sums/broadcasts (GPSIMD stays on its default ucode lib)
  3. indirect-DMA gather of the winning row spread over 16 partitions x 64
     (row appears twice); masked iota-dot -> column -cx
  4. ACT: colsq=(j-cx)^2, rowsq=(r-cy)^2, dist_t=sqrt(colsq+rowsq_t)
  5. z = x*dist: chunk t=0 fused mul+accum on DVE -> partials[:, b];
     chunks t=1..3 multiplied on GPSIMD, column-summed into one PSUM row by
     accumulating PE matmuls (lhsT = ones column)
Host sums partials + colsums in float64 and divides by b.
"""

import numpy as np

B_FULL = 128
H = 512
W = 512
N_CORES = 8
B_CORE = B_FULL // N_CORES  # 16 images per core
T = 4                       # rows per partition
ROWP = 32                   # partitions for the gathered-row scan
ROWF = 16                   # free elems per partition in the row scan

_CACHE = {}


def _consts():
    cols = np.broadcast_to(np.arange(W, dtype=np.float32), (128, W)).copy()
    r4c = (4 * np.arange(128, dtype=np.float32)[:, None]
           + np.arange(T, dtype=np.float32)[None, :]).copy()
    r4enc8 = (32.0 * (4096.0 - r4c)).copy()
    lin = (ROWF * np.arange(ROWP, dtype=np.float32)[:, None]
           + np.arange(ROWF, dtype=np.float32)[None, :])
    colenc = (512.0 - lin).astype(np.float32)
    pidx8 = np.arange(ROWP, dtype=np.float32)[:, None].copy()
    ones2 = np.ones((1, 2), dtype=np.float32)
    pec = np.concatenate([np.eye(128, dtype=np.float32),
                          np.ones((128, 128), dtype=np.float32)], axis=1)
    return {"COLS": cols, "R4C": r4c, "R4ENC8": r4enc8, "COLENC": colenc,
            "PIDX8": pidx8, "ONES2": ones2, "PEC": pec}


def build_program(debug=False, b_core=None, no_indirect=False,
                  no_psum_acc=False, repeat=1):
    import concourse.bass as bass
    import concourse.bacc as bacc
    import concourse.mybir as mybir
    from concourse.tile import TileContext

    nb = b_core or B_CORE
    f32 = mybir.dt.float32
    u32 = mybir.dt.uint32
    Alu = mybir.AluOpType
    Act = mybir.ActivationFunctionType

    nc = bacc.Bacc("TRN2", target_bir_lowering=False, debug=False)

    x_d = nc.dram_tensor("x", [nb, H, W], f32, kind="ExternalInput")
    cols_d = nc.dram_tensor("COLS", [128, W], f32, kind="ExternalInput")
    r4c_d = nc.dram_tensor("R4C", [128, T], f32, kind="ExternalInput")
    r4enc8_d = nc.dram_tensor("R4ENC8", [128, T], f32, kind="ExternalInput")
    pidx8_d = nc.dram_tensor("PIDX8", [ROWP, 1], f32, kind="ExternalInput")
    colenc_d = nc.dram_tensor("COLENC", [ROWP, ROWF], f32, kind="ExternalInput")
    ones2_d = nc.dram_tensor("ONES2", [1, 2], f32, kind="ExternalInput")
    pec_d = nc.dram_tensor("PEC", [128, 256], f32, kind="ExternalInput")
    out_d = nc.dram_tensor("partials", [128, 2 * nb], f32, kind="ExternalOutput")
    csum_d = nc.dram_tensor("colsums", [1, W], f32, kind="ExternalOutput")
    if debug:
        dbg_d = nc.dram_tensor("dbg", [1, 4 * nb], f32, kind="ExternalOutput")

    x_ap = x_d.ap()
    x_rows = x_ap.rearrange("b h (s i) -> (b h s) i", i=ROWF)  # 64-elem sub-rows

    with TileContext(nc) as tc:
        with (
            tc.tile_pool(name="consts", bufs=1) as consts,
            tc.tile_pool(name="xs", bufs=8) as xs,
            tc.tile_pool(name="work", bufs=8) as work,
            tc.tile_pool(name="small", bufs=8) as small,
            tc.tile_pool(name="rows", bufs=8) as rows,
            tc.tile_pool(name="ps", bufs=3, space="PSUM") as ps,
            tc.tile_pool(name="pacc", bufs=1, space="PSUM") as pacc,
        ):
            cols_t = consts.tile([128, W], f32)
            nc.sync.dma_start(out=cols_t, in_=cols_d.ap())
            r4c_t = consts.tile([128, T], f32)
            nc.sync.dma_start(out=r4c_t, in_=r4c_d.ap())
            r4enc8_t = consts.tile([128, T], f32)
            nc.sync.dma_start(out=r4enc8_t, in_=r4enc8_d.ap())
            pidx8_t = consts.tile([ROWP, 1], f32)
            nc.sync.dma_start(out=pidx8_t, in_=pidx8_d.ap())
            colenc_t = consts.tile([ROWP, ROWF], f32)
            nc.sync.dma_start(out=colenc_t, in_=colenc_d.ap())
            ones2_t = consts.tile([1, 2], f32)
            nc.sync.dma_start(out=ones2_t, in_=ones2_d.ap())
            pec_t = consts.tile([128, 256], f32)
            nc.sync.dma_start(out=pec_t, in_=pec_d.ap())
            ident_t = pec_t[:, 0:128]
            ones_t = pec_t[:, 128:256]

            # prime PE on the const tile so later PE ops carry only one wait
            prime_ps = ps.tile([1, 128], f32, tag="tpose")
            nc.tensor.transpose(prime_ps, pec_t[:, 128:129], ident_t)

            partials_t = consts.tile([128, 2 * nb], f32)
            colsums_ps = pacc.tile([1, W], f32)
            if debug:
                dbg_t = consts.tile([1, 4 * nb], f32)
                nc.vector.memset(dbg_t, 0.0)

            def s0(st):
                b = st["b"]
                x_t = xs.tile([128, T, W], f32)
                xv = x_ap[b].rearrange("(p t) w -> p t w", p=128)
                rowmax = small.tile([128, T], f32)
                for q in range(4):
                    nc.sync.dma_start(out=x_t[:, q:q + 1, :],
                                      in_=xv[:, q:q + 1, :])
                    nc.vector.reduce_max(rowmax[:, q:q + 1], x_t[:, q, :],
                                         axis=mybir.AxisListType.X)
                pmax = small.tile([128, 1], f32)
                nc.vector.reduce_max(pmax, rowmax, axis=mybir.AxisListType.X)
                pmax_ps = ps.tile([1, 128], f32, tag="tpose")
                nc.tensor.transpose(pmax_ps, pmax, ident_t)
                st.update(x_t=x_t, rowmax=rowmax, pmax_ps=pmax_ps)

            def s1(st):
                m_sb = small.tile([1, 1], f32)
                nc.vector.reduce_max(m_sb, st["pmax_ps"],
                                     axis=mybir.AxisListType.X)
                sc2 = ps.tile([128, 2], f32, tag="sc")
                nc.tensor.matmul(sc2[:, 0:1], ones_t[0:1, :], m_sb)
                msb128 = small.tile([128, 1], f32)
                nc.vector.tensor_copy(msb128, sc2[:, 0:1])
                # locate row: max of (rowmax==M)*(32*(4096-r))
                e4 = small.tile([128, T], f32)
                junk1 = small.tile([128, 1], f32)
                nc.vector.scalar_tensor_tensor(
                    e4, st["rowmax"], sc2[:, 0:1], r4enc8_t,
                    op0=Alu.is_equal, op1=Alu.mult, accum_out=junk1)
                epmax = small.tile([128, 1], f32)
                nc.vector.reduce_max(epmax, e4, axis=mybir.AxisListType.X)
                ep_ps = ps.tile([1, 128], f32, tag="tpose")
                nc.tensor.transpose(ep_ps, epmax, ident_t)
                st.update(sc2=sc2, ep_ps=ep_ps, msb128=msb128)

            def s2(st):
                b = st["b"]
                e_sb = small.tile([1, 1], f32)
                nc.vector.reduce_max(e_sb, st["ep_ps"],
                                     axis=mybir.AxisListType.X)
                sc2 = st["sc2"]
                nc.tensor.matmul(sc2[:, 1:2], ones_t[0:1, :], e_sb)
                # gather offsets: p + 32*cy + 32*512*b
                cyu = small.tile([ROWP, 1], u32)
                nc.vector.tensor_scalar(
                    cyu, pidx8_t, sc2[0:ROWP, 1:2],
                    float(32 * 4096 + ROWP * H * b),
                    op0=Alu.subtract, op1=Alu.add)
                negcy_sb = small.tile([128, 1], f32)
                nc.vector.tensor_scalar(
                    negcy_sb, sc2[:, 1:2], 1.0 / 32.0, -4096.0,
                    op0=Alu.mult, op1=Alu.add)
                st.update(cyu=cyu, negcy_sb=negcy_sb)

            def s3(st):
                b = st["b"]
                rowbuf = rows.tile([ROWP, ROWF], f32)
                if no_indirect:
                    nc.sync.dma_start(out=rowbuf, in_=x_ap[b, 0:ROWP, 0:ROWF])
                else:
                    nc.gpsimd.indirect_dma_start(
                        out=rowbuf,
                        out_offset=None,
                        in_=x_rows,
                        in_offset=bass.IndirectOffsetOnAxis(
                            ap=st["cyu"][:], axis=0),
                    )
                st.update(rowbuf=rowbuf)

            def s4(st):
                # locate column: max of (row==M)*(512-j), DVE-only 32-max
                ohr = rows.tile([ROWP, ROWF], f32)
                junkr = small.tile([ROWP, 1], f32)
                nc.vector.scalar_tensor_tensor(
                    ohr, st["rowbuf"], st["msb128"][0:ROWP, 0:1], colenc_t,
                    op0=Alu.is_equal, op1=Alu.mult, accum_out=junkr)
                ecp = small.tile([ROWP, 1], f32)
                nc.vector.reduce_max(ecp, ohr, axis=mybir.AxisListType.X)
                ecb = small.tile([ROWP, ROWP], f32)
                nc.vector.tensor_scalar(
                    ecb, ident_t[0:ROWP, 0:ROWP], 0.0, ecp,
                    op0=Alu.mult, op1=Alu.add)
                ec32 = small.tile([ROWP, 1], f32)
                nc.vector.tensor_reduce(
                    ec32, ecb, axis=mybir.AxisListType.X,
                    op=Alu.max, apply_transpose=True)
                scx = ps.tile([128, 2], f32, tag="sc")
                nc.tensor.matmul(scx[:, 0:1], ones_t[0:1, :], ec32[0:1, 0:1])
                negcx_sb = small.tile([128, 1], f32)
                nc.vector.tensor_scalar(
                    negcx_sb, scx[:, 0:1], -512.0, None, op0=Alu.add)
                st.update(negcx_sb=negcx_sb)
                if debug:
                    b = st["b"]
                    nc.vector.tensor_scalar(
                        dbg_t[0:1, 4 * b:4 * b + 1], ones2_t[0:1, 0:1],
                        st["msb128"][0:1, 0:1], None, op0=Alu.mult)
                    nc.vector.tensor_scalar(
                        dbg_t[0:1, 4 * b + 1:4 * b + 2], ones2_t[0:1, 0:1],
                        st["negcy_sb"][0:1, 0:1], None, op0=Alu.mult)
                    nc.vector.tensor_scalar(
                        dbg_t[0:1, 4 * b + 2:4 * b + 3], ones2_t[0:1, 0:1],
                        negcx_sb[0:1, 0:1], None, op0=Alu.mult)

            def s5(st):
                x_t = st["x_t"]
                colsq = work.tile([128, W], f32)
                nc.scalar.activation(colsq, cols_t, Act.Square,
                                     bias=st["negcx_sb"])
                rowsq = small.tile([128, T], f32)
                nc.scalar.activation(rowsq, r4c_t, Act.Square,
                                     bias=st["negcy_sb"])
                dists = []
                zs = []
                for t in range(T):
                    dist_t = work.tile([128, W], f32, tag="dist")
                    nc.scalar.activation(
                        dist_t, colsq, Act.Sqrt, bias=rowsq[:, t:t + 1])
                    dists.append(dist_t)
                    if t > 0:
                        z_t = work.tile([128, W], f32, tag="zg")
                        hi = W if t < T - 1 else W // 2
                        nc.gpsimd.tensor_tensor(
                            z_t[:, 0:hi], dist_t[:, 0:hi],
                            x_t[:, t, 0:hi], op=Alu.mult)
                        zs.append(z_t)
                st.update(dist0=dists[0], dist3=dists[T - 1], zs=zs)

            def s6(st, first, last):
                x_t, b = st["x_t"], st["b"]
                z_t = work.tile([128, W], f32, tag="zd")
                nc.vector.scalar_tensor_tensor(
                    z_t, st["dist0"], 1.0, x_t[:, 0, :],
                    op0=Alu.mult, op1=Alu.mult,
                    accum_out=partials_t[:, 2 * b:2 * b + 1])
                # DVE covers the half of chunk T-1 that Pool skipped
                zh = work.tile([128, W // 2], f32, tag="zh")
                nc.vector.scalar_tensor_tensor(
                    zh, st["dist3"][:, W // 2:W], 1.0,
                    x_t[:, T - 1, W // 2:W],
                    op0=Alu.mult, op1=Alu.mult,
                    accum_out=partials_t[:, 2 * b + 1:2 * b + 2])
                for t in range(1, T):
                    hi = W if t < T - 1 else W // 2
                    nc.tensor.matmul(
                        colsums_ps[0:1, 0:hi], ones_t[:, 0:1],
                        st["zs"][t - 1][:, 0:hi],
                        start=(no_psum_acc or (first and t == 1)),
                        stop=(no_psum_acc or (last and t == T - 1)),
                        skip_group_check=True)

            stages = [s0, s1, s2, s3, s4, s5]
            todo = [(rep, b) for rep in range(repeat) for b in range(nb)]
            n = len(todo)
            states = {}
            NS = 7
            for i in range(n + NS - 1):
                # oldest stage first so each sequencer sees only ready work
                j = i - (NS - 1)
                if 0 <= j < n:
                    s6(states.pop(j), first=(j == 0), last=(j == n - 1))
                for k in range(NS - 2, -1, -1):
                    j = i - k
                    if 0 <= j < n:
                        if k == 0:
                            states[j] = {"b": todo[j][1], "rep": todo[j][0]}
                        stages[k](states[j])

            colsums_sb = consts.tile([1, W], f32)
            nc.vector.tensor_copy(colsums_sb, colsums_ps)
            nc.sync.dma_start(out=out_d.ap(), in_=partials_t)
            nc.sync.dma_start(out=csum_d.ap(), in_=colsums_sb)
            if debug:
                nc.sync.dma_start(out=dbg_d.ap(), in_=dbg_t)

    nc.compile()
    return nc


def kernel(x: np.ndarray) -> np.ndarray:
    from concourse import bass_utils

    key = "nc"
    if key not in _CACHE:
        _CACHE[key] = build_program(debug=False)
    nc = _CACHE[key]

    x = np.ascontiguousarray(x, dtype=np.float32)
    shards = x.reshape(N_CORES, B_CORE, H, W)
    consts = _consts()
    in_maps = [dict(consts, x=shards[i]) for i in range(N_CORES)]
    res = bass_utils.run_bass_kernel_spmd(
        nc, in_maps, core_ids=list(range(N_CORES)))
    total = 0.0
    for r in res.results:
        total += r["partials"].astype(np.float64).sum()
        total += r["colsums"].astype(np.float64).sum()
    return np.float32(total / B_FULL)
